# revision 1
# baseline (speedup 1.0000x reference)
"""Trainium2 Bass kernel for nn_EquivariantPerturbationTransform.

Reference computation (N=6000 genes, D=256, H=8 heads, P=128 perturbations,
B=16 batches):
  q = H @ Wq.T ; k,v from gathered perturbation rows
  scores[h,n,p] shared across batches; per-batch mask over p (ragged)
  attn_out[b] = softmax-masked attention -> out proj (zeroed for empty batches)
  x = LN1(H + attn_out); out = LN2(x + gelu(x@W1.T)@W2.T)

Strategy:
  - Sequence-parallel over 8 cores: N padded to 6144, 768 query rows/core,
    all B batches per core. H_genes/params replicated (small), so per-core
    HBM traffic is ~2.5MB of inputs + 12.3MB output.
  - batch_assignment is sorted -> each batch owns a CONTIGUOUS p-range. The
    128 perturbations are cut into eight 16-row blocks; a block-diagonal
    value matrix V_bd[(h,p16), (slot,h',e)] per block turns the masked
    per-batch, per-head attention*V contraction into plain 128-K matmuls
    producing row-layout context for one batch-slot at a time (never
    materializes [B,h,N,P] logits, no partition-offset matmul outputs --
    fp32r matmuls only accept column tile-position 0 on this compiler).
  - exp() without max-subtraction: scores are O(10) here so fp32 exp is safe
    (ratios exact); softmax denominators via one mask-matmul per head,
    transposed per-head to row layout [n, (h,b)] and applied as a
    broadcast-AP multiply; empty batches get +1 denominators and skip
    attention entirely at program-build time.
  - fp32r everywhere on the PE (full 1 cycle/row at moving-N>=256, ~1e-4
    matmul error vs 2e-3 for bf16); fp32 accumulation in PSUM.
"""

import os
import sys

sys.path.insert(0, "/opt/trn_rl_repo")

import numpy as np

import concourse.bass as bass
from concourse import mybir
from concourse.tile import TileContext

F32 = mybir.dt.float32
F32R = mybir.dt.float32r
AF = mybir.ActivationFunctionType

N, D, H, P, B = 6000, 256, 8, 128, 16
DH = D // H  # 32
NCORES = 8
NPAD = 6144          # 8 * 768
NG = NPAD // NCORES  # 768 rows per core
NT = NG // 128       # 6 row-tiles per core
NCH = 2              # moving-dim chunks
CH = NG // NCH       # 384 (>=256 keeps fp32r at full rate)
EPS = 1e-5
GW = 16              # perturbation block width
NGRP = P // GW       # 8 blocks


def _split_waits(nc, max_waits=1):
    """The neuronxcc/walrus build in this container rejects instructions with
    more than one sync-wait condition. Hoist excess waits onto NoOps injected
    just before, on the same engine (semantically identical)."""
    n_split = 0
    for f in nc.m.functions:
        for bb in f.blocks:
            new_list = []
            for ins in bb.instructions:
                si = getattr(ins, "sync_info", None)
                if si is not None and si.on_wait and len(si.on_wait) > max_waits:
                    waits = list(si.on_wait)
                    excess, keep = waits[:-max_waits], waits[-max_waits:]
                    for i in range(0, len(excess), max_waits):
                        chunk = excess[i : i + max_waits]
                        nop = mybir.InstNoOp(name=f"{ins.name}-ws{i}", ins=[], outs=[])
                        nop.engine = ins.engine
                        nop.sync_info = mybir.SyncInfo(on_wait=chunk, on_update=[])
                        new_list.append(nop)
                        n_split += 1
                    si.on_wait = keep
                new_list.append(ins)
            bb.instructions = new_list
    return n_split


def _build_program(counts, groups, contribs, flags):
    """Build the per-core SPMD Bass program.

    groups[g]   = list of (b, p_lo, p_len) for batches intersecting block g
    contribs[b] = list of (g, slot_idx) covering batch b's p-range
    """
    (use_bq, use_bk, use_bv, use_bo, use_b1, use_b2,
     use_g1, use_b1ln, use_g2, use_b2ln) = flags
    nc = bass.Bass()

    # ---- DRAM parameters -------------------------------------------------
    hg_row = nc.declare_dram_parameter("hg_row", [NG, D], F32, isOutput=False)
    hg_t = nc.declare_dram_parameter("hg_t", [D, NG], F32R, isOutput=False)
    hp_t = nc.declare_dram_parameter("hp_t", [D, P], F32R, isOutput=False)
    m01bd = nc.declare_dram_parameter("m01bd", [NGRP, 128, 128], F32R, isOutput=False)
    emptyp = nc.declare_dram_parameter("emptyp", [128, 1], F32, isOutput=False)
    ident = nc.declare_dram_parameter("ident", [128, 128], F32, isOutput=False)
    identr = nc.declare_dram_parameter("identr", [128, 128], F32R, isOutput=False)
    identb = nc.declare_dram_parameter("identb", [128, 32], F32, isOutput=False)
    smax = max(1, max(len(g) for g in groups))
    zeros_r = nc.declare_dram_parameter("zeros_r", [128, smax * D], F32R, isOutput=False)
    wq_t = nc.declare_dram_parameter("wq_t", [D, D], F32R, isOutput=False)
    wk_t = nc.declare_dram_parameter("wk_t", [D, D], F32R, isOutput=False)
    wv_t = nc.declare_dram_parameter("wv_t", [D, D], F32R, isOutput=False)
    wo_t = nc.declare_dram_parameter("wo_t", [D, D], F32R, isOutput=False)
    w1_t = nc.declare_dram_parameter("w1_t", [D, 4 * D], F32R, isOutput=False)
    w2_t = nc.declare_dram_parameter("w2_t", [4 * D, D], F32R, isOutput=False)
    bias_kv = nc.declare_dram_parameter("bias_kv", [D, 2], F32, isOutput=False)
    bq_col = nc.declare_dram_parameter("bq_col", [D, 1], F32, isOutput=False)
    bo_mask = nc.declare_dram_parameter("bo_mask", [D, B], F32, isOutput=False)
    b1_col = nc.declare_dram_parameter("b1_col", [4 * D, 1], F32, isOutput=False)
    b2_col = nc.declare_dram_parameter("b2_col", [D, 1], F32, isOutput=False)
    ln1_col = nc.declare_dram_parameter("ln1_col", [D, 2], F32, isOutput=False)
    gb_row = nc.declare_dram_parameter("gb_row", [4, D], F32, isOutput=False)
    out = nc.declare_dram_parameter("out", [B, NG, D], F32, isOutput=True)

    s_attn = 1.0 / float(np.sqrt(DH))

    with TileContext(nc) as tc, nc.allow_low_precision(
            reason="fp32r is a deliberate rounding of matmul inputs"):
        import contextlib

        cstack = contextlib.ExitStack()
        consts = cstack.enter_context(tc.tile_pool(name="consts", bufs=1))

        # ---- load constants -------------------------------------------
        hgr_sb = []
        for t in range(NT):
            tl = consts.tile([128, D], F32, tag=f"hgr{t}", name=f"hgr{t}")
            nc.sync.dma_start(out=tl[:], in_=hg_row[t * 128 : (t + 1) * 128, :])
            hgr_sb.append(tl)
        hgt_sb = []
        for kk in range(2):
            tl = consts.tile([128, NG], F32R, tag=f"hgt{kk}", name=f"hgt{kk}")
            nc.sync.dma_start(out=tl[:], in_=hg_t[kk * 128 : (kk + 1) * 128, :])
            hgt_sb.append(tl)
        hpt_sb = []
        for kk in range(2):
            tl = consts.tile([128, P], F32R, tag=f"hpt{kk}", name=f"hpt{kk}")
            nc.sync.dma_start(out=tl[:], in_=hp_t[kk * 128 : (kk + 1) * 128, :])
            hpt_sb.append(tl)

        def load_w(name, ap, rows, cols, dt=F32):
            tiles = []
            for kk in range(rows // 128):
                tl = consts.tile([128, cols], dt, tag=f"{name}{kk}", name=f"{name}{kk}")
                nc.sync.dma_start(out=tl[:], in_=ap[kk * 128 : (kk + 1) * 128, :])
                tiles.append(tl)
            return tiles

        wq_sb = load_w("wq", wq_t, D, D, dt=F32R)
        wk_sb = load_w("wk", wk_t, D, D, dt=F32R)
        wv_sb = load_w("wv", wv_t, D, D, dt=F32R)
        wo_sb = load_w("wo", wo_t, D, D, dt=F32R)
        w1_sb = load_w("w1", w1_t, D, 4 * D, dt=F32R)
        w2_sb = load_w("w2", w2_t, 4 * D, D, dt=F32R)

        m01bd_sb = []
        for g in range(NGRP):
            tl = consts.tile([128, 128], F32R, tag=f"m01bd{g}", name=f"m01bd{g}")
            nc.sync.dma_start(out=tl[:], in_=m01bd[g, :, :])
            m01bd_sb.append(tl)
        empty_sb = consts.tile([128, 1], F32, tag="empty", name="empty")
        nc.sync.dma_start(out=empty_sb[:], in_=emptyp[:, :])
        ident_sb = consts.tile([128, 128], F32, tag="ident", name="ident")
        nc.sync.dma_start(out=ident_sb[:], in_=ident[:, :])
        identr_sb = consts.tile([128, 128], F32R, tag="identr", name="identr")
        nc.sync.dma_start(out=identr_sb[:], in_=identr[:, :])
        # per-32-block identity so transposes of partition-offset slices can
        # use an identity operand starting at the same partition
        identb_sb = consts.tile([128, 32], F32, tag="identb", name="identb")
        nc.sync.dma_start(out=identb_sb[:], in_=identb[:, :])
        eps_sb = consts.tile([128, 1], F32, tag="eps", name="eps")
        nc.vector.memset(eps_sb[:], EPS)

        bkv_sb = None
        if use_bk or use_bv:
            bkv_sb = load_w("bkv", bias_kv, D, 2)
        bq_sb = load_w("bq", bq_col, D, 1) if use_bq else None
        bo_sb = load_w("bo", bo_mask, D, B) if use_bo else None
        b1_sb = load_w("b1", b1_col, 4 * D, 1) if use_b1 else None
        b2_sb = load_w("b2", b2_col, D, 1) if use_b2 else None
        # ln1 affine as [D,1] columns for the T-layout residual (general path)
        ln1_sb = load_w("ln1c", ln1_col, D, 2) if (use_g1 or use_b1ln) else None
        # broadcast [1,D] rows across 128 partitions for general ln2-affine /
        # v-bias paths (skipped when trivial)
        gbr_sb = None
        if use_g2 or use_b2ln or use_bv:
            gbr_sb = consts.tile([128, 4, D], F32, tag="gbr", name="gbr")
            nc.gpsimd.dma_start(out=gbr_sb[:], in_=gb_row[:, :].to_broadcast((128, 4, D)))

        # persistent activation tiles
        qT_sb = [consts.tile([128, NG], F32R, tag=f"qT{i}", name=f"qT{i}") for i in range(2)]
        kT_sb = [consts.tile([128, P], F32R, tag=f"kT{i}", name=f"kT{i}") for i in range(2)]
        v_sb = consts.tile([P, D], F32R, tag="v", name="v")
        # E regrouped per perturbation block: Eg[g][(h, p16), n]
        Eg = [consts.tile([128, NG], F32R, tag=f"Eg{g}", name=f"Eg{g}")
              for g in range(NGRP)]
        # block-diagonal masked values: vbd[g][(h, p16), (slot, h', e)]
        vbd = [consts.tile([128, max(1, len(groups[g])) * D], F32R,
                           tag=f"vbd{g}", name=f"vbd{g}") for g in range(NGRP)]
        # softmax denominators: packed [(h,b), n], then row layout [n, (h,b)]
        denp = consts.tile([128, NG], F32, tag="denp", name="denp")
        rden_row = consts.tile([128, NT, 128], F32, tag="rden_row", name="rden_row")

        # ================= Phase A: shared projections ==================
        with tc.tile_pool(name="psA", bufs=4, space="PSUM") as psA, \
             tc.tile_pool(name="psD", bufs=2, space="PSUM") as psD, \
             tc.tile_pool(name="etpool", bufs=1) as etpool:
            Et = etpool.tile([128, H, NG], F32R, tag="Et", name="Et")
            # qT [D, NG] = Wq^T-stationary applied to hg_t
            for m in range(2):
                for c in range(NCH):
                    ps = psA.tile([128, CH], F32, tag="ps", name="ps")
                    for kk in range(2):
                        nc.tensor.matmul(
                            ps[:],
                            wq_sb[kk][:, m * 128 : (m + 1) * 128],
                            hgt_sb[kk][:, c * CH : (c + 1) * CH],
                            start=(kk == 0), stop=(kk == 1),
                        )
                    if use_bq:
                        nc.scalar.activation(
                            qT_sb[m][:, c * CH : (c + 1) * CH], ps[:], AF.Identity,
                            bias=bq_sb[m][:, 0:1])
                    else:
                        nc.scalar.activation(
                            qT_sb[m][:, c * CH : (c + 1) * CH], ps[:], AF.Copy)

            # kT [D, P]
            for m in range(2):
                ps = psA.tile([128, P], F32, tag="ps", name="ps")
                for kk in range(2):
                    nc.tensor.matmul(
                        ps[:], wk_sb[kk][:, m * 128 : (m + 1) * 128],
                        hpt_sb[kk][:], start=(kk == 0), stop=(kk == 1))
                if use_bk:
                    nc.scalar.activation(kT_sb[m][:], ps[:], AF.Identity,
                                         bias=bkv_sb[m][:, 0:1])
                else:
                    nc.scalar.activation(kT_sb[m][:], ps[:], AF.Copy)

            # v row-layout [P, D]
            ps_v = psA.tile([P, D], F32, tag="ps", name="psv")
            for kk in range(2):
                nc.tensor.matmul(ps_v[:], hpt_sb[kk][:], wv_sb[kk][:],
                                 start=(kk == 0), stop=(kk == 1))
            if use_bv:
                nc.vector.tensor_add(v_sb[:], ps_v[:], gbr_sb[:P, 3, :])
            else:
                nc.vector.tensor_copy(out=v_sb[:], in_=ps_v[:])

            # E^T per head: exp(s * k_h @ q_h^T)  -> Et[p, h, n]
            for h in range(H):
                kt = kT_sb[h // 4]
                for c in range(NCH):
                    ps = psA.tile([128, CH], F32, tag="ps", name="ps")
                    nc.tensor.matmul(
                        ps[:],
                        kt[(h % 4) * DH : (h % 4 + 1) * DH, :],
                        qT_sb[h // 4][(h % 4) * DH : (h % 4 + 1) * DH,
                                      c * CH : (c + 1) * CH],
                        start=True, stop=True,
                        tile_position=((h % 4) * DH, 0))
                    nc.scalar.activation(Et[:, h, c * CH : (c + 1) * CH],
                                         ps[:], AF.Exp, scale=s_attn)

            # regroup E into per-block layout (partition moves via DMA)
            for g in range(NGRP):
                for h in range(H):
                    nc.sync.dma_start(
                        out=Eg[g][h * GW : (h + 1) * GW, :],
                        in_=Et[g * GW : (g + 1) * GW, h, :])

            # block-diagonal masked values (zero-fill via DMA: memset can't
            # write fp32r-typed tiles on this compiler)
            for g in range(NGRP):
                if groups[g]:
                    nc.sync.dma_start(
                        out=vbd[g][:],
                        in_=zeros_r[:, : len(groups[g]) * D])
                for s, (b, p_lo, p_len) in enumerate(groups[g]):
                    for h in range(H):
                        po = p_lo - g * GW
                        nc.sync.dma_start(
                            out=vbd[g][h * GW + po : h * GW + po + p_len,
                                       s * D + h * DH : s * D + (h + 1) * DH],
                            in_=v_sb[p_lo : p_lo + p_len,
                                     h * DH : (h + 1) * DH])

            # denominators, packed layout: denp[(h,b), n] via block-diagonal
            # mask matmuls accumulated over the 8 perturbation blocks
            for c in range(NCH):
                psd = psD.tile([128, CH], F32, tag="psd", name="psd")
                for g in range(NGRP):
                    nc.tensor.matmul(
                        psd[:], m01bd_sb[g][:],
                        Eg[g][:, c * CH : (c + 1) * CH],
                        start=(g == 0), stop=(g == NGRP - 1))
                # +1 on empty batches so the reciprocal is finite
                nc.scalar.activation(
                    denp[:, c * CH : (c + 1) * CH],
                    psd[:], AF.Identity, bias=empty_sb[:, 0:1])
            nc.vector.reciprocal(out=denp[:], in_=denp[:])
            # transpose reciprocal denominators to row layout [n, (h,b)],
            # two heads (32 rows) per transpose to stay 32-aligned
            for t in range(NT):
                for hp in range(4):
                    psr = psD.tile([128, 32], F32, tag="psd", name="psr")
                    nc.tensor.transpose(
                        psr[:], denp[hp * 32 : (hp + 1) * 32,
                                     t * 128 : (t + 1) * 128],
                        identb_sb[hp * 32 : (hp + 1) * 32, :],
                        tile_position=(hp * 32, 0))
                    nc.scalar.activation(
                        rden_row[:, t, hp * 32 : (hp + 1) * 32],
                        psr[:], AF.Copy)

        # ================= Phase B: per-batch back half =================
        work = cstack.enter_context(tc.tile_pool(name="work", bufs=2))
        h1pool = cstack.enter_context(tc.tile_pool(name="h1p", bufs=1))
        ps_mm = cstack.enter_context(tc.tile_pool(name="ps_mm", bufs=4, space="PSUM"))
        ps_tr = cstack.enter_context(tc.tile_pool(name="ps_tr", bufs=4, space="PSUM"))

        for b in range(B):
            Lb = int(counts[b]) if b < len(counts) else 0
            ctxT = None
            if Lb > 0:
                # --- attention context: block-diag matmuls give row-layout
                # ctx per batch; normalize with broadcast-AP multiply; PE
                # transpose into [(h,e), n] for the projection.
                ctxT = work.tile([128, 2, NG], F32R, tag="ctxT", name="ctxT")
                for t in range(NT):
                    psc = ps_mm.tile([128, D], F32, tag="mm", name="mmc")
                    cl = contribs[b]
                    for i, (g, s) in enumerate(cl):
                        nc.tensor.matmul(
                            psc[:],
                            Eg[g][:, t * 128 : (t + 1) * 128],
                            vbd[g][:, s * D : (s + 1) * D],
                            start=(i == 0), stop=(i == len(cl) - 1))
                    # multiply by 1/denom[n, h] (free-dim broadcast over e)
                    rr = rden_row[:, t, :]
                    rbc = bass.AP(tensor=rr.tensor, offset=rr.offset + b,
                                  ap=[rr.ap[0], [GW, H], [0, DH]])
                    ctxr = work.tile([128, H, DH], F32R, tag="ctxr", name="ctxr")
                    nc.vector.tensor_mul(
                        ctxr[:],
                        psc[:].rearrange("p (h e) -> p h e", h=H), rbc)
                    pst = ps_tr.tile([128, D], F32R, tag="tr", name="trc")
                    for m in range(2):
                        nc.tensor.transpose(
                            pst[:, m * 128 : (m + 1) * 128],
                            ctxr[:, :, :].rearrange("p h e -> p (h e)")[
                                :, m * 128 : (m + 1) * 128],
                            identr_sb[:])
                    for m in range(2):
                        nc.scalar.activation(
                            ctxT[:, m, t * 128 : (t + 1) * 128],
                            pst[:, m * 128 : (m + 1) * 128].bitcast(F32), AF.Copy)

                # --- out-projection (transposed) ---
                aoT = work.tile([128, 2, NG], F32, tag="aoT", name="aoT")
                for m in range(2):
                    for c in range(NCH):
                        ps = ps_mm.tile([128, CH], F32, tag="mm", name="mm")
                        for kk in range(2):
                            nc.tensor.matmul(
                                ps[:],
                                wo_sb[kk][:, m * 128 : (m + 1) * 128],
                                ctxT[:, kk, c * CH : (c + 1) * CH],
                                start=(kk == 0), stop=(kk == 1))
                        if use_bo:
                            nc.scalar.activation(
                                aoT[:, m, c * CH : (c + 1) * CH], ps[:], AF.Identity,
                                bias=bo_sb[m][:, b : b + 1])
                        else:
                            nc.scalar.activation(
                                aoT[:, m, c * CH : (c + 1) * CH], ps[:], AF.Copy)

            # --- residual + LN1 (row layout), re-transpose to xT ---
            xT = work.tile([128, 2, NG], F32R, tag="xT", name="xT")
            for t in range(NT):
                r1 = work.tile([128, D], F32, tag="r1", name="r1")
                if Lb > 0:
                    pst = ps_tr.tile([128, D], F32, tag="tr", name="tr")
                    for m in range(2):
                        nc.tensor.transpose(
                            pst[:, m * 128 : (m + 1) * 128],
                            aoT[:, m, t * 128 : (t + 1) * 128], ident_sb[:])
                    nc.vector.tensor_add(r1[:], pst[:], hgr_sb[t][:])
                else:
                    nc.vector.tensor_copy(out=r1[:], in_=hgr_sb[t][:])

                stats = work.tile([128, 6], F32, tag="stats", name="stats")
                mv = work.tile([128, 2], F32, tag="mv", name="mv")
                nc.vector.bn_stats(out=stats[:], in_=r1[:])
                nc.vector.bn_aggr(out=mv[:], in_=stats[:])
                nc.scalar.activation(mv[:, 1:2], mv[:, 1:2], AF.Sqrt,
                                     bias=eps_sb[:, 0:1])
                nc.vector.reciprocal(out=mv[:, 1:2], in_=mv[:, 1:2])
                # xr = xhat (unit-affine LN); g1/b1_ln are folded into the FFN
                # weights on host, and applied per-partition in T-layout for
                # the residual below when nontrivial.
                xr = work.tile([128, D], F32, tag="xr", name="xr")
                nc.vector.tensor_scalar(
                    out=xr[:], in0=r1[:], scalar1=mv[:, 0:1], scalar2=mv[:, 1:2],
                    op0=mybir.AluOpType.subtract, op1=mybir.AluOpType.mult)
                pst2 = ps_tr.tile([128, D], F32, tag="tr", name="tr")
                for m in range(2):
                    nc.tensor.transpose(
                        pst2[:, m * 128 : (m + 1) * 128],
                        xr[:, m * 128 : (m + 1) * 128], ident_sb[:])
                for m in range(2):
                    nc.scalar.activation(
                        xT[:, m, t * 128 : (t + 1) * 128],
                        pst2[:, m * 128 : (m + 1) * 128], AF.Copy)

            # --- FFN1 + exact gelu ---
            h1g = h1pool.tile([128, 8, NG], F32R, tag="h1g", name="h1g")
            for m in range(8):
                for c in range(NCH):
                    ps = ps_mm.tile([128, CH], F32, tag="mm", name="mm")
                    for kk in range(2):
                        nc.tensor.matmul(
                            ps[:],
                            w1_sb[kk][:, m * 128 : (m + 1) * 128],
                            xT[:, kk, c * CH : (c + 1) * CH],
                            start=(kk == 0), stop=(kk == 1))
                    if use_b1:
                        nc.scalar.activation(h1g[:, m, c * CH : (c + 1) * CH],
                                             ps[:], AF.Gelu,
                                             bias=b1_sb[m][:, 0:1])
                    else:
                        nc.scalar.activation(h1g[:, m, c * CH : (c + 1) * CH],
                                             ps[:], AF.Gelu)

            # --- FFN2 + residual -> yT ---
            # residual adds x_ln = xhat*g1 + b1_ln; per-partition affine in
            # T-layout when the ln1 affine is nontrivial, else xT directly.
            if use_g1 or use_b1ln:
                xres = work.tile([128, 2, NG], F32, tag="xres", name="xres")
                for m in range(2):
                    nc.vector.tensor_scalar(
                        out=xres[:, m, :], in0=xT[:, m, :].bitcast(F32),
                        scalar1=ln1_sb[m][:, 0:1], scalar2=ln1_sb[m][:, 1:2],
                        op0=mybir.AluOpType.mult, op1=mybir.AluOpType.add)
            else:
                xres = xT
            yT = work.tile([128, 2, NG], F32, tag="yT", name="yT")
            for m in range(2):
                for c in range(NCH):
                    ps = ps_mm.tile([128, CH], F32, tag="mm", name="mm")
                    for kk in range(8):
                        nc.tensor.matmul(
                            ps[:],
                            w2_sb[kk][:, m * 128 : (m + 1) * 128],
                            h1g[:, kk, c * CH : (c + 1) * CH],
                            start=(kk == 0), stop=(kk == 7))
                    if use_b2:
                        nc.vector.scalar_tensor_tensor(
                            out=yT[:, m, c * CH : (c + 1) * CH], in0=ps[:],
                            scalar=b2_sb[m][:, 0:1],
                            in1=xres[:, m, c * CH : (c + 1) * CH].bitcast(F32),
                            op0=mybir.AluOpType.add, op1=mybir.AluOpType.add)
                    else:
                        nc.vector.tensor_add(
                            yT[:, m, c * CH : (c + 1) * CH], ps[:],
                            xres[:, m, c * CH : (c + 1) * CH].bitcast(F32))

            # --- LN2 (row layout) + store ---
            for t in range(NT):
                psy = ps_tr.tile([128, D], F32, tag="tr", name="tr")
                for m in range(2):
                    nc.tensor.transpose(
                        psy[:, m * 128 : (m + 1) * 128],
                        yT[:, m, t * 128 : (t + 1) * 128], ident_sb[:])
                yr = work.tile([128, D], F32, tag="yr", name="yr")
                nc.scalar.activation(yr[:], psy[:], AF.Copy)
                stats = work.tile([128, 6], F32, tag="stats", name="stats")
                mv = work.tile([128, 2], F32, tag="mv", name="mv")
                nc.vector.bn_stats(out=stats[:], in_=yr[:])
                nc.vector.bn_aggr(out=mv[:], in_=stats[:])
                nc.scalar.activation(mv[:, 1:2], mv[:, 1:2], AF.Sqrt,
                                     bias=eps_sb[:, 0:1])
                nc.vector.reciprocal(out=mv[:, 1:2], in_=mv[:, 1:2])
                orow = work.tile([128, D], F32, tag="orow", name="orow")
                nc.vector.tensor_scalar(
                    out=orow[:], in0=yr[:], scalar1=mv[:, 0:1], scalar2=mv[:, 1:2],
                    op0=mybir.AluOpType.subtract, op1=mybir.AluOpType.mult)
                if use_g2:
                    nc.vector.tensor_mul(orow[:], orow[:], gbr_sb[:, 2, :])
                if use_b2ln:
                    nc.vector.tensor_add(orow[:], orow[:], gbr_sb[:, 3, :])
                nc.sync.dma_start(out=out[b, t * 128 : (t + 1) * 128, :],
                                  in_=orow[:])

        cstack.close()

    return nc


def kernel(H_genes, perturbation_indices, batch_assignment, batch_size,
           in_proj_w, in_proj_b, out_proj_w, out_proj_b,
           ffn_w1, ffn_b1, ffn_w2, ffn_b2,
           ln1_g, ln1_b, ln2_g, ln2_b):
    Hg = np.ascontiguousarray(np.asarray(H_genes, dtype=np.float32))
    pidx = np.asarray(perturbation_indices).astype(np.int64)
    ba = np.asarray(batch_assignment).astype(np.int64)
    Bs = int(np.asarray(batch_size))
    assert Bs == B, f"kernel hardcodes B=16, got {Bs}"
    assert Hg.shape == (N, D)

    Wq, Wk, Wv = [np.asarray(w, np.float32) for w in np.split(np.asarray(in_proj_w), 3, axis=0)]
    bq, bk, bv = [np.asarray(x, np.float32) for x in np.split(np.asarray(in_proj_b), 3, axis=0)]
    Wo = np.asarray(out_proj_w, np.float32)
    bo = np.asarray(out_proj_b, np.float32)
    W1 = np.asarray(ffn_w1, np.float32)
    b1 = np.asarray(ffn_b1, np.float32)
    W2 = np.asarray(ffn_w2, np.float32)
    b2 = np.asarray(ffn_b2, np.float32)
    g1 = np.asarray(ln1_g, np.float32)
    be1 = np.asarray(ln1_b, np.float32)
    g2 = np.asarray(ln2_g, np.float32)
    be2 = np.asarray(ln2_b, np.float32)

    # ragged batch ranges (batch_assignment is sorted)
    counts = np.bincount(ba, minlength=B).astype(np.int64)
    starts = np.concatenate([[0], np.cumsum(counts)[:-1]]).astype(np.int64)
    has_any = (counts > 0)

    # block/slot decomposition of the sorted p-ranges
    groups = []
    for g in range(NGRP):
        lo, hi = g * GW, (g + 1) * GW
        sl = []
        for b in range(B):
            s, e = int(starts[b]), int(starts[b] + counts[b])
            s2, e2 = max(s, lo), min(e, hi)
            if s2 < e2:
                sl.append((b, s2, e2 - s2))
        groups.append(sl)
    contribs = {b: [] for b in range(B)}
    for g in range(NGRP):
        for s, (b, _, _) in enumerate(groups[g]):
            contribs[b].append((g, s))

    # fold ln1 affine into FFN1 (exact): W1' = W1*g1, b1' = W1@b1_ln + b1
    W1f = W1 * g1[None, :]
    b1f = b1 + W1 @ be1

    Hp = np.ascontiguousarray(Hg[pidx])             # [P, D]
    Hg_pad = np.zeros((NPAD, D), np.float32)
    Hg_pad[:N] = Hg

    m01 = (ba[:, None] == np.arange(16)[None, :]).astype(np.float32)
    m01bd = np.zeros((NGRP, 128, 128), np.float32)
    for g in range(NGRP):
        for h in range(H):
            m01bd[g, h * GW : (h + 1) * GW, h * GW : (h + 1) * GW] = \
                m01[g * GW : (g + 1) * GW, :]
    emptyp = np.tile((~has_any).astype(np.float32), H)[:, None]
    ident = np.eye(128, dtype=np.float32)
    bo_mask = (bo[:, None] * has_any[None, :].astype(np.float32))  # [D, B]
    gb_row = np.stack([g1, be1, g2, be2], axis=0)                  # [4, D]

    flags = (
        bool(np.any(bq != 0)), bool(np.any(bk != 0)), bool(np.any(bv != 0)),
        bool(np.any(bo != 0)), bool(np.any(b1f != 0)), bool(np.any(b2 != 0)),
        bool(np.any(g1 != 1)), bool(np.any(be1 != 0)),
        bool(np.any(g2 != 1)), bool(np.any(be2 != 0)),
    )

    nc = _build_program(counts, groups, contribs, flags)

    common = {
        "hp_t": np.ascontiguousarray(Hp.T),
        "m01bd": m01bd,
        "emptyp": np.ascontiguousarray(emptyp),
        "ident": ident,
        "identr": ident,
        "identb": np.ascontiguousarray(np.tile(np.eye(32, dtype=np.float32), (4, 1))),
        "wq_t": np.ascontiguousarray(Wq.T),
        "wk_t": np.ascontiguousarray(Wk.T),
        "wv_t": np.ascontiguousarray(Wv.T),
        "wo_t": np.ascontiguousarray(Wo.T),
        "w1_t": np.ascontiguousarray(W1f.T),
        "w2_t": np.ascontiguousarray(W2.T),
        "bias_kv": np.ascontiguousarray(np.stack([bk, bv], axis=1)),
        "bq_col": bq[:, None].copy(),
        "bo_mask": np.ascontiguousarray(bo_mask),
        "b1_col": b1f[:, None].copy(),
        "b2_col": b2[:, None].copy(),
        "ln1_col": np.ascontiguousarray(np.stack([g1, be1], axis=1)),
        "gb_row": gb_row,
        "zeros_r": np.zeros((128, max(1, max(len(g) for g in groups)) * D), np.float32),
    }
    in_maps = []
    for c in range(NCORES):
        sl = Hg_pad[c * NG : (c + 1) * NG]
        m = dict(common)
        m["hg_row"] = np.ascontiguousarray(sl)
        m["hg_t"] = np.ascontiguousarray(sl.T)
        in_maps.append(m)

    if os.environ.get("BASS_KERNEL_SIM"):
        from concourse import bass_interp
        # CoreSim lacks a Gelu implementation; shim in exact (erf) gelu for
        # local debugging (HW uses the ACT LUT).
        if not getattr(bass_interp.InstructionExecutor, "_gelu_patched", False):
            from scipy.special import erf
            _orig_act = bass_interp.InstructionExecutor.visit_InstActivation

            def _act(self, instruction, *, reg_snapshot=None):
                if instruction.func == mybir.ActivationFunctionType.Gelu:
                    instruction.func = mybir.ActivationFunctionType.Identity
                    try:
                        import concourse.bass_interp as bi
                        out_ap = instruction.outs[0]
                        r = _orig_act(self, instruction, reg_snapshot=reg_snapshot)
                        view = self.view_ap(out_ap, bi.Direction.READ, instruction,
                                            reg_snapshot=reg_snapshot)
                        x = view.astype(np.float64)
                        view[:] = (0.5 * x * (1.0 + erf(x / np.sqrt(2.0)))).astype(view.dtype)
                        return r
                    finally:
                        instruction.func = mybir.ActivationFunctionType.Gelu
                return _orig_act(self, instruction, reg_snapshot=reg_snapshot)

            bass_interp.InstructionExecutor.visit_InstActivation = _act
            bass_interp.InstructionExecutor._gelu_patched = True
        nsim = int(os.environ.get("BASS_KERNEL_SIM_CORES", "1"))
        simtrace = bool(os.environ.get("BASS_KERNEL_SIMTRACE"))
        sim = bass_interp.MultiCoreSim(nc, nsim, trace=simtrace)
        for c in range(nsim):
            for k, v in in_maps[c].items():
                sim.cores[c].tensor(k)[:] = v
        sim.simulate()
        print(f"SIM predicted time: {sim.cores[0].time} ns")
        full = np.zeros((B, NPAD, D), np.float32)
        for c in range(nsim):
            full[:, c * NG : (c + 1) * NG, :] = (
                np.array(sim.cores[c].mem_tensor("out")).reshape(B, NG, D))
        return full[:, :N, :]

    from concourse.bass_utils import run_bass_kernel_spmd
    _split_waits(nc)
    trace = bool(os.environ.get("BASS_KERNEL_TRACE"))
    res = run_bass_kernel_spmd(nc, in_maps, core_ids=list(range(NCORES)),
                               trace=trace)
    if trace and res.exec_time_ns is not None:
        print(f"HW exec time: {res.exec_time_ns} ns")
        if res.instructions_and_trace:
            print("trace:", res.instructions_and_trace[1])

    full = np.zeros((B, NPAD, D), np.float32)
    for c in range(NCORES):
        full[:, c * NG : (c + 1) * NG, :] = res.results[c]["out"]
    return full[:, :N, :]



# revision 12
# speedup vs baseline: 1.3356x; 1.3356x over previous
"""Trainium2 Bass kernel for nn_EquivariantPerturbationTransform.

Reference computation (N=6000 genes, D=256, H=8 heads, P=128 perturbations,
B=16 batches):
  q = H @ Wq.T ; k,v from gathered perturbation rows
  scores[h,n,p] shared across batches; per-batch mask over p (ragged)
  attn_out[b] = softmax-masked attention -> out proj (zeroed for empty batches)
  x = LN1(H + attn_out); out = LN2(x + gelu(x@W1.T)@W2.T)

Strategy (v2, bf16):
  - Sequence-parallel over 8 cores: N padded to 6144, NG=768 query rows/core,
    all B batches per core; H_genes/params replicated.
  - All matmul operands bf16 (FWL fast weight loads; fp32 stationaries
    disable FWL and made LDWEIGHTS ~40% of runtime in the fp32r version).
    PSUM accumulation stays fp32. Tolerance is 2e-2; bf16 keeps us ~5e-3.
  - k/v are tiny (P=128 rows): computed on host. Wo and bo are folded into
    per-head value vectors Vt[(h,p)] = V_h[p] @ Wo_h^T + bo/H, so attention
    context IS the attention output, directly in transposed layout:
      aoT[e,n] = sum_{h,p} (E[(h,p),n]/den[h,n]) * Vt[(h,p),e]
    via block-diagonal stationaries over the 8 contiguous 16-row
    perturbation blocks (batch_assignment is sorted).
  - Per-batch softmax denominators are produced directly in the broadcast
    layout [(h,p16),n] by a 0/1 block-diag mask matmul (columns (h,q)
    replicate the batch's row-sum for every q), so normalizing E is one
    vector multiply; no partition-broadcast DMA needed.
  - LayerNorm row trips via PE transposes (bf16). rstd = 1/sqrt(var+eps)
    is computed on the vector engine with the Quake III bit-trick seed + 2
    Newton steps; the scalar engine only ever runs Exp (phase A) and Gelu,
    so the 1.3us ACT_TABLE_LOads happen twice total instead of ~66x.
  - Output written bf16 (halves store traffic), widened to fp32 on host.
"""

import os
import sys

sys.path.insert(0, "/opt/trn_rl_repo")

import numpy as np
import ml_dtypes

import concourse.bass as bass
from concourse import mybir
from concourse.tile import TileContext

F32 = mybir.dt.float32
BF16 = mybir.dt.bfloat16
I32 = mybir.dt.int32
AF = mybir.ActivationFunctionType
ALU = mybir.AluOpType

N, D, H, P, B = 6000, 256, 8, 128, 16
DH = D // H  # 32
NCORES = 8
NPAD = 6144          # 8 * 768
NG = NPAD // NCORES  # 768 rows per core
NT = NG // 128       # 6 row-tiles per core
NCH = 2              # moving-dim chunks (matmul psum must fit a 2KB bank)
CH = NG // NCH       # 384
EPS = 1e-5
GW = 16              # perturbation block width
NGRP = P // GW       # 8 blocks
QMAGIC = 0x5F3759DF  # quake rsqrt seed magic


def _split_waits(nc, max_waits=1):
    """The neuronxcc/walrus build in this container rejects instructions with
    more than one sync-wait condition. Hoist excess waits onto NoOps injected
    just before, on the same engine (semantically identical)."""
    n_split = 0
    for f in nc.m.functions:
        for bb in f.blocks:
            new_list = []
            for ins in bb.instructions:
                si = getattr(ins, "sync_info", None)
                if si is not None and si.on_wait and len(si.on_wait) > max_waits:
                    waits = list(si.on_wait)
                    excess, keep = waits[:-max_waits], waits[-max_waits:]
                    for i in range(0, len(excess), max_waits):
                        chunk = excess[i : i + max_waits]
                        nop = mybir.InstNoOp(name=f"{ins.name}-ws{i}", ins=[], outs=[])
                        nop.engine = ins.engine
                        nop.sync_info = mybir.SyncInfo(on_wait=chunk, on_update=[])
                        new_list.append(nop)
                        n_split += 1
                    si.on_wait = keep
                new_list.append(ins)
            bb.instructions = new_list
    return n_split


def _build_program(counts, contribs, nsel, smax, smaxc, flags):
    """Build the per-core SPMD Bass program.

    contribs[b] = list of (sel_idx, g, s): batch b's attention sums over
    perturbation block g using vbd slot s, with selbg[sel_idx] the matching
    denominator mask.
    """
    (use_bq, use_b1, use_b2, use_g1, use_b1ln, use_g2, use_b2ln) = flags
    nc = bass.Bass()

    # ---- DRAM parameters -------------------------------------------------
    hg_t = nc.declare_dram_parameter("hg_t", [D, NG], BF16, isOutput=False)
    kt = nc.declare_dram_parameter("kt", [D, P], BF16, isOutput=False)
    wq_t = nc.declare_dram_parameter("wq_t", [D, D], BF16, isOutput=False)
    w1_t = nc.declare_dram_parameter("w1_t", [D, 4 * D], BF16, isOutput=False)
    w2_t = nc.declare_dram_parameter("w2_t", [4 * D, D], BF16, isOutput=False)
    vbdp = nc.declare_dram_parameter("vbdp", [NGRP, 128, smax * D], BF16, isOutput=False)
    selbg = nc.declare_dram_parameter("selbg", [max(nsel, 1), 128, 128], BF16, isOutput=False)
    ident = nc.declare_dram_parameter("ident", [128, 128], BF16, isOutput=False)
    bq_col = nc.declare_dram_parameter("bq_col", [D, 1], F32, isOutput=False)
    b1_col = nc.declare_dram_parameter("b1_col", [4 * D, 1], F32, isOutput=False)
    b2_col = nc.declare_dram_parameter("b2_col", [D, 1], F32, isOutput=False)
    ln1_col = nc.declare_dram_parameter("ln1_col", [D, 2], F32, isOutput=False)
    gb_row = nc.declare_dram_parameter("gb_row", [2, D], F32, isOutput=False)
    out = nc.declare_dram_parameter("out", [B, NG, D], BF16, isOutput=True)

    s_attn = 1.0 / float(np.sqrt(DH))

    with TileContext(nc) as tc, nc.allow_low_precision(
            reason="bf16 matmuls/activations are a deliberate precision trade"):
        import contextlib

        cstack = contextlib.ExitStack()
        consts = cstack.enter_context(tc.tile_pool(name="consts", bufs=1))

        # ---- load constants -------------------------------------------
        hgt_sb = []
        for kk in range(2):
            tl = consts.tile([128, NG], BF16, tag=f"hgt{kk}", name=f"hgt{kk}")
            nc.sync.dma_start(out=tl[:], in_=hg_t[kk * 128 : (kk + 1) * 128, :])
            hgt_sb.append(tl)

        def load_w(name, ap, rows, cols, dt=BF16):
            tiles = []
            for kk in range(rows // 128):
                tl = consts.tile([128, cols], dt, tag=f"{name}{kk}", name=f"{name}{kk}")
                nc.sync.dma_start(out=tl[:], in_=ap[kk * 128 : (kk + 1) * 128, :])
                tiles.append(tl)
            return tiles

        wq_sb = load_w("wq", wq_t, D, D)
        kt_sb = load_w("kt", kt, D, P)
        w1_sb = load_w("w1", w1_t, D, 4 * D)
        w2_sb = load_w("w2", w2_t, 4 * D, D)

        vbd_sb = []
        for g in range(NGRP):
            tl = consts.tile([128, smax * D], BF16, tag=f"vbd{g}", name=f"vbd{g}")
            nc.sync.dma_start(out=tl[:], in_=vbdp[g, :, :])
            vbd_sb.append(tl)
        sel_sb = []
        for i in range(nsel):
            tl = consts.tile([128, 128], BF16, tag=f"sel{i}", name=f"sel{i}")
            eng = (nc.sync, nc.gpsimd)[i % 2]
            eng.dma_start(out=tl[:], in_=selbg[i, :, :])
            sel_sb.append(tl)
        ident_sb = consts.tile([128, 128], BF16, tag="ident", name="ident")
        nc.sync.dma_start(out=ident_sb[:], in_=ident[:, :])

        magic_sb = consts.tile([128, NT], I32, tag="magic", name="magic")
        nc.vector.memset(magic_sb[:], QMAGIC)

        bq_sb = load_w("bq", bq_col, D, 1, dt=F32) if use_bq else None
        b1_sb = load_w("b1", b1_col, 4 * D, 1, dt=F32) if use_b1 else None
        b2_sb = load_w("b2", b2_col, D, 1, dt=F32) if use_b2 else None
        ln1_sb = load_w("ln1c", ln1_col, D, 2, dt=F32) if (use_g1 or use_b1ln) else None
        gbr_sb = None
        if use_g2 or use_b2ln:
            gbr_sb = consts.tile([128, 2, D], F32, tag="gbr", name="gbr")
            nc.gpsimd.dma_start(out=gbr_sb[:], in_=gb_row[:, :].to_broadcast((128, 2, D)))

        # persistent activation tiles
        qT_sb = [consts.tile([128, NG], BF16, tag=f"qT{i}", name=f"qT{i}") for i in range(2)]
        Et = consts.tile([128, H, NG], BF16, tag="Et", name="Et")
        Eg = [consts.tile([128, NG], BF16, tag=f"Eg{g}", name=f"Eg{g}")
              for g in range(NGRP)]

        # ================= Phase A: shared projections ==================
        with tc.tile_pool(name="psA", bufs=4, space="PSUM") as psA:
            # qT [D, NG] = Wq^T-stationary applied to hg_t
            for m in range(2):
                for c in range(NCH):
                    ps = psA.tile([128, CH], F32, tag="ps", name="ps")
                    for kk in range(2):
                        nc.tensor.matmul(
                            ps[:],
                            wq_sb[kk][:, m * 128 : (m + 1) * 128],
                            hgt_sb[kk][:, c * CH : (c + 1) * CH],
                            start=(kk == 0), stop=(kk == 1),
                        )
                    if use_bq:
                        nc.scalar.activation(
                            qT_sb[m][:, c * CH : (c + 1) * CH], ps[:],
                            AF.Identity, bias=bq_sb[m][:, 0:1])
                    else:
                        nc.vector.tensor_copy(
                            out=qT_sb[m][:, c * CH : (c + 1) * CH], in_=ps[:])

            # E^T per head: exp(s * k_h @ q_h^T)  -> Et[p, h, n]
            for h in range(H):
                for c in range(NCH):
                    ps = psA.tile([128, CH], F32, tag="ps", name="ps")
                    nc.tensor.matmul(
                        ps[:],
                        kt_sb[h // 4][(h % 4) * DH : (h % 4 + 1) * DH, :],
                        qT_sb[h // 4][(h % 4) * DH : (h % 4 + 1) * DH,
                                      c * CH : (c + 1) * CH],
                        start=True, stop=True,
                        tile_position=((h % 4) * DH, 0))
                    nc.scalar.activation(Et[:, h, c * CH : (c + 1) * CH],
                                         ps[:], AF.Exp, scale=s_attn)

            # regroup E into per-block layout (partition moves via DMA)
            for g in range(NGRP):
                for h in range(H):
                    eng = (nc.sync, nc.gpsimd)[(g * H + h) % 2]
                    eng.dma_start(
                        out=Eg[g][h * GW : (h + 1) * GW, :],
                        in_=Et[g * GW : (g + 1) * GW, h, :])

        # ================= Phase B: per-batch back half =================
        work = cstack.enter_context(tc.tile_pool(name="work", bufs=2))
        h1pool = cstack.enter_context(tc.tile_pool(name="h1p", bufs=1))
        ps_mm = cstack.enter_context(tc.tile_pool(name="ps_mm", bufs=2, space="PSUM"))
        ps_row = cstack.enter_context(tc.tile_pool(name="ps_row", bufs=1, space="PSUM"))
        ps_xt = cstack.enter_context(tc.tile_pool(name="ps_xt", bufs=1, space="PSUM"))

        def rsqrt_quake(pool, veps, tagp):
            """rstd [128, NT] = 1/sqrt(veps) on DVE, no activation tables.

            Quake III bit trick seed (|rel err| <= 3.5% for any positive
            float), then two Newton steps -> ~5e-6.
            """
            seed = pool.tile([128, NT], F32, tag=f"{tagp}_seed", name=f"{tagp}_seed")
            nc.vector.tensor_scalar(
                out=seed[:].bitcast(I32), in0=veps[:].bitcast(I32),
                scalar1=1, scalar2=None, op0=ALU.logical_shift_right)
            nc.vector.tensor_tensor(
                out=seed[:].bitcast(I32), in0=magic_sb[:],
                in1=seed[:].bitcast(I32), op=ALU.subtract)
            r = seed
            for it in range(2):
                a = pool.tile([128, NT], F32, tag=f"{tagp}_nr{it}", name=f"{tagp}_nr{it}")
                # a = r*r ; a = (a * -0.5) * veps ; r = (a + 1.5) * r
                nc.vector.tensor_tensor(out=a[:], in0=r[:], in1=r[:], op=ALU.mult)
                nc.vector.scalar_tensor_tensor(
                    out=a[:], in0=a[:], scalar=-0.5, in1=veps[:],
                    op0=ALU.mult, op1=ALU.mult)
                rn = pool.tile([128, NT], F32, tag=f"{tagp}_r{it}", name=f"{tagp}_r{it}")
                nc.vector.scalar_tensor_tensor(
                    out=rn[:], in0=a[:], scalar=1.5, in1=r[:],
                    op0=ALU.add, op1=ALU.mult)
                r = rn
            return r

        def layernorm_rows(src_tiles, pool, tagp):
            """Transpose T-layout x into rows, return (ps_rowt, mvb, rstd).

            src_tiles: two [128, NG]-shaped APs (m-blocks of features).
            Emits: 2*NT transposes into a [128, NT, D] bf16 PSUM tile,
            bn_stats/aggr per tile, veps, and the quake rsqrt. The PSUM tile
            tag is shared between LN1/LN2 (bufs=1 ring keeps PSUM <= 16KB).
            """
            psr = pool.tile([128, NT, D], BF16, tag="lnr_psr", name=f"{tagp}_psr")
            for t in range(NT):
                for m in range(2):
                    nc.tensor.transpose(
                        psr[:, t, m * 128 : (m + 1) * 128],
                        src_tiles[m][:, t * 128 : (t + 1) * 128],
                        ident_sb[:])
            stats = work.tile([128, NT, 6], F32, tag=f"{tagp}_st", name=f"{tagp}_st")
            mvb = work.tile([128, NT, 2], F32, tag=f"{tagp}_mv", name=f"{tagp}_mv")
            for t in range(NT):
                nc.vector.bn_stats(out=stats[:, t, :], in_=psr[:, t, :])
                nc.vector.bn_aggr(out=mvb[:, t, :], in_=stats[:, t, :])
            veps = work.tile([128, NT], F32, tag=f"{tagp}_ve", name=f"{tagp}_ve")
            nc.vector.tensor_scalar(
                out=veps[:], in0=mvb[:, :, 1], scalar1=EPS, scalar2=None,
                op0=ALU.add)
            rstd = rsqrt_quake(work, veps, tagp)
            return psr, mvb, rstd

        for b in range(B):
            Lb = int(counts[b]) if b < len(counts) else 0
            cl = contribs[b]

            if Lb > 0:
                # --- denominators, already broadcast to [(h,p16), n] ---
                denb = work.tile([128, NG], BF16, tag="denb", name="denb")
                for c in range(NCH):
                    psd = ps_mm.tile([128, CH], F32, tag="mm", name="psd")
                    for i, (si, g, s) in enumerate(cl):
                        nc.tensor.matmul(
                            psd[:], sel_sb[si][:],
                            Eg[g][:, c * CH : (c + 1) * CH],
                            start=(i == 0), stop=(i == len(cl) - 1))
                    nc.vector.reciprocal(
                        out=denb[:, c * CH : (c + 1) * CH], in_=psd[:])

                # --- normalized E per contributing block ---
                egb = work.tile([128, smaxc, NG], BF16, tag="egb", name="egb")
                for i, (si, g, s) in enumerate(cl):
                    nc.vector.tensor_tensor(
                        out=egb[:, i, :], in0=Eg[g][:], in1=denb[:], op=ALU.mult)

                # --- attention output, directly transposed [(m,e),n] ---
                xpreT = work.tile([128, 2, NG], BF16, tag="xpreT", name="xpreT")
                for m in range(2):
                    for c in range(NCH):
                        psa = ps_mm.tile([128, CH], F32, tag="mm", name="mma")
                        for i, (si, g, s) in enumerate(cl):
                            nc.tensor.matmul(
                                psa[:],
                                vbd_sb[g][:, s * D + m * 128 : s * D + (m + 1) * 128],
                                egb[:, i, c * CH : (c + 1) * CH],
                                start=(i == 0), stop=(i == len(cl) - 1))
                        # residual: x^T = H^T + ao^T
                        nc.vector.tensor_tensor(
                            out=xpreT[:, m, c * CH : (c + 1) * CH], in0=psa[:],
                            in1=hgt_sb[m][:, c * CH : (c + 1) * CH],
                            op=ALU.add)
                xsrc = [xpreT[:, 0, :], xpreT[:, 1, :]]
            else:
                # empty perturbation set: attention output is exactly zero
                xsrc = [hgt_sb[0][:], hgt_sb[1][:]]

            # --- LN1 (row trip via PE transposes) ---
            psr1, mv1, rstd1 = layernorm_rows(xsrc, ps_row, "ln1")

            # xhat rows -> back to T layout for the FFN
            xhatT = work.tile([128, 2, NG], BF16, tag="xhatT", name="xhatT")
            psx = ps_xt.tile([128, NT, D], BF16, tag="psx", name="psx")
            for t in range(NT):
                xr = work.tile([128, D], BF16, tag="xr", name="xr")
                nc.vector.tensor_scalar(
                    out=xr[:], in0=psr1[:, t, :],
                    scalar1=mv1[:, t, 0:1], scalar2=rstd1[:, t : t + 1],
                    op0=ALU.subtract, op1=ALU.mult)
                for m in range(2):
                    nc.tensor.transpose(
                        psx[:, t, m * 128 : (m + 1) * 128],
                        xr[:, m * 128 : (m + 1) * 128],
                        ident_sb[:])
            for m in range(2):
                nc.vector.tensor_copy(
                    out=xhatT[:, m, :],
                    in_=psx[:, :, m * 128 : (m + 1) * 128])

            # residual operand for LN2: xhat with the (folded) ln1 affine
            if use_g1 or use_b1ln:
                xresT = work.tile([128, 2, NG], BF16, tag="xresT", name="xresT")
                for m in range(2):
                    nc.vector.tensor_scalar(
                        out=xresT[:, m, :], in0=xhatT[:, m, :],
                        scalar1=ln1_sb[m][:, 0:1], scalar2=ln1_sb[m][:, 1:2],
                        op0=ALU.mult, op1=ALU.add)
            else:
                xresT = xhatT

            # --- FFN1 + exact gelu ---
            h1g = h1pool.tile([128, 8, NG], BF16, tag="h1g", name="h1g")
            for m in range(8):
                for c in range(NCH):
                    ps = ps_mm.tile([128, CH], F32, tag="mm", name="mm")
                    for kk in range(2):
                        nc.tensor.matmul(
                            ps[:],
                            w1_sb[kk][:, m * 128 : (m + 1) * 128],
                            xhatT[:, kk, c * CH : (c + 1) * CH],
                            start=(kk == 0), stop=(kk == 1))
                    if use_b1:
                        nc.scalar.activation(
                            h1g[:, m, c * CH : (c + 1) * CH], ps[:], AF.Gelu,
                            bias=b1_sb[m][:, 0:1])
                    else:
                        nc.scalar.activation(
                            h1g[:, m, c * CH : (c + 1) * CH], ps[:], AF.Gelu)

            # --- FFN2 + residual -> yT ---
            yT = work.tile([128, 2, NG], BF16, tag="yT", name="yT")
            for m in range(2):
                for c in range(NCH):
                    ps = ps_mm.tile([128, CH], F32, tag="mm", name="mm")
                    for kk in range(8):
                        nc.tensor.matmul(
                            ps[:],
                            w2_sb[kk][:, m * 128 : (m + 1) * 128],
                            h1g[:, kk, c * CH : (c + 1) * CH],
                            start=(kk == 0), stop=(kk == 7))
                    if use_b2:
                        nc.vector.scalar_tensor_tensor(
                            out=yT[:, m, c * CH : (c + 1) * CH], in0=ps[:],
                            scalar=b2_sb[m][:, 0:1],
                            in1=xresT[:, m, c * CH : (c + 1) * CH],
                            op0=ALU.add, op1=ALU.add)
                    else:
                        nc.vector.tensor_tensor(
                            out=yT[:, m, c * CH : (c + 1) * CH], in0=ps[:],
                            in1=xresT[:, m, c * CH : (c + 1) * CH],
                            op=ALU.add)

            # --- LN2 (row trip) + store ---
            psr2, mv2, rstd2 = layernorm_rows(
                [yT[:, 0, :], yT[:, 1, :]], ps_row, "ln2")
            orows = work.tile([128, NT, D], BF16, tag="orows", name="orows")
            for t in range(NT):
                nc.vector.tensor_scalar(
                    out=orows[:, t, :], in0=psr2[:, t, :],
                    scalar1=mv2[:, t, 0:1], scalar2=rstd2[:, t : t + 1],
                    op0=ALU.subtract, op1=ALU.mult)
                if use_g2:
                    nc.vector.tensor_tensor(
                        out=orows[:, t, :], in0=orows[:, t, :],
                        in1=gbr_sb[:, 0, :], op=ALU.mult)
                if use_b2ln:
                    nc.vector.tensor_tensor(
                        out=orows[:, t, :], in0=orows[:, t, :],
                        in1=gbr_sb[:, 1, :], op=ALU.add)
            # one store per batch: out[b, t*128+p, d] <- orows[p, t, d]
            nc.sync.dma_start(
                out=out[b].rearrange("(t p) d -> p t d", p=128),
                in_=orows[:])

        cstack.close()

    return nc


def _host_prep(H_genes, perturbation_indices, batch_assignment,
               in_proj_w, in_proj_b, out_proj_w, out_proj_b,
               ffn_w1, ffn_b1, ffn_w2, ffn_b2,
               ln1_g, ln1_b, ln2_g, ln2_b):
    Hg = np.ascontiguousarray(np.asarray(H_genes, dtype=np.float32))
    pidx = np.asarray(perturbation_indices).astype(np.int64)
    ba = np.asarray(batch_assignment).astype(np.int64)

    Wq, Wk, Wv = [np.asarray(w, np.float32) for w in np.split(np.asarray(in_proj_w), 3, axis=0)]
    bq, bk, bv = [np.asarray(x, np.float32) for x in np.split(np.asarray(in_proj_b), 3, axis=0)]
    Wo = np.asarray(out_proj_w, np.float32)
    bo = np.asarray(out_proj_b, np.float32)
    W1 = np.asarray(ffn_w1, np.float32)
    b1 = np.asarray(ffn_b1, np.float32)
    W2 = np.asarray(ffn_w2, np.float32)
    b2 = np.asarray(ffn_b2, np.float32)
    g1 = np.asarray(ln1_g, np.float32)
    be1 = np.asarray(ln1_b, np.float32)
    g2 = np.asarray(ln2_g, np.float32)
    be2 = np.asarray(ln2_b, np.float32)

    # ragged batch ranges (batch_assignment is sorted)
    counts = np.bincount(ba, minlength=B).astype(np.int64)
    starts = np.concatenate([[0], np.cumsum(counts)[:-1]]).astype(np.int64)

    # block/slot decomposition of the sorted p-ranges
    groups = []
    for g in range(NGRP):
        lo, hi = g * GW, (g + 1) * GW
        sl = []
        for b in range(B):
            s, e = int(starts[b]), int(starts[b] + counts[b])
            s2, e2 = max(s, lo), min(e, hi)
            if s2 < e2:
                sl.append((b, s2, e2 - s2))
        groups.append(sl)
    smax = max(1, max(len(g) for g in groups))

    # host-side k and folded values: Vt[(h,p)] = V_h[p] @ Wo_h^T + bo/H
    Hp = Hg[pidx]                                   # [P, D]
    k = Hp @ Wk.T + bk                              # [P, D]
    V = Hp @ Wv.T + bv                              # [P, D]
    vbdp = np.zeros((NGRP, 128, smax * D), np.float32)
    for h in range(H):
        Voh = V[:, h * DH : (h + 1) * DH] @ Wo[:, h * DH : (h + 1) * DH].T \
            + bo[None, :] / H                       # [P, D]
        for g in range(NGRP):
            for s, (b, p_lo, p_len) in enumerate(groups[g]):
                po = p_lo - g * GW
                vbdp[g, h * GW + po : h * GW + po + p_len,
                     s * D : (s + 1) * D] = Voh[p_lo : p_lo + p_len, :]

    # denominator masks: selbg[i][(h,p16),(h,q)] = 1 iff row g*16+p in batch b
    contribs = {b: [] for b in range(B)}
    sel_list = []
    for g in range(NGRP):
        for s, (b, p_lo, p_len) in enumerate(groups[g]):
            po = p_lo - g * GW
            m = np.zeros((128, 128), np.float32)
            for h in range(H):
                m[h * GW + po : h * GW + po + p_len,
                  h * GW : (h + 1) * GW] = 1.0
            contribs[b].append((len(sel_list), g, s))
            sel_list.append(m)
    nsel = len(sel_list)
    selbg = (np.stack(sel_list, axis=0) if nsel
             else np.zeros((1, 128, 128), np.float32))
    smaxc = max(1, max(len(c) for c in contribs.values()))

    # fold ln1 affine into FFN1 (exact): W1' = W1*g1, b1' = W1@b1_ln + b1
    W1f = W1 * g1[None, :]
    b1f = b1 + W1 @ be1

    Hg_pad = np.zeros((NPAD, D), np.float32)
    Hg_pad[:N] = Hg

    flags = (
        bool(np.any(bq != 0)), bool(np.any(b1f != 0)), bool(np.any(b2 != 0)),
        bool(np.any(g1 != 1)), bool(np.any(be1 != 0)),
        bool(np.any(g2 != 1)), bool(np.any(be2 != 0)),
    )

    bf = ml_dtypes.bfloat16
    common = {
        "kt": np.ascontiguousarray(k.T).astype(bf),
        "wq_t": np.ascontiguousarray(Wq.T).astype(bf),
        "w1_t": np.ascontiguousarray(W1f.T).astype(bf),
        "w2_t": np.ascontiguousarray(W2.T).astype(bf),
        "vbdp": vbdp.astype(bf),
        "selbg": selbg.astype(bf),
        "ident": np.eye(128, dtype=np.float32).astype(bf),
        "bq_col": bq[:, None].copy(),
        "b1_col": b1f[:, None].copy(),
        "b2_col": b2[:, None].copy(),
        "ln1_col": np.ascontiguousarray(np.stack([g1, be1], axis=1)),
        "gb_row": np.ascontiguousarray(np.stack([g2, be2], axis=0)),
    }
    in_maps = []
    for c in range(NCORES):
        sl = Hg_pad[c * NG : (c + 1) * NG]
        m = dict(common)
        m["hg_t"] = np.ascontiguousarray(sl.T).astype(bf)
        in_maps.append(m)
    return counts, contribs, nsel, smax, smaxc, flags, in_maps


def kernel(H_genes, perturbation_indices, batch_assignment, batch_size,
           in_proj_w, in_proj_b, out_proj_w, out_proj_b,
           ffn_w1, ffn_b1, ffn_w2, ffn_b2,
           ln1_g, ln1_b, ln2_g, ln2_b):
    Bs = int(np.asarray(batch_size))
    assert Bs == B, f"kernel hardcodes B=16, got {Bs}"
    assert np.asarray(H_genes).shape == (N, D)

    counts, contribs, nsel, smax, smaxc, flags, in_maps = _host_prep(
        H_genes, perturbation_indices, batch_assignment,
        in_proj_w, in_proj_b, out_proj_w, out_proj_b,
        ffn_w1, ffn_b1, ffn_w2, ffn_b2, ln1_g, ln1_b, ln2_g, ln2_b)

    nc = _build_program(counts, contribs, nsel, smax, smaxc, flags)

    if os.environ.get("BASS_KERNEL_SIM"):
        from concourse import bass_interp
        # CoreSim lacks a Gelu implementation; shim in exact (erf) gelu for
        # local debugging (HW uses the ACT LUT).
        if not getattr(bass_interp.InstructionExecutor, "_gelu_patched", False):
            from scipy.special import erf
            _orig_act = bass_interp.InstructionExecutor.visit_InstActivation

            def _act(self, instruction, *, reg_snapshot=None):
                if instruction.func == mybir.ActivationFunctionType.Gelu:
                    instruction.func = mybir.ActivationFunctionType.Identity
                    try:
                        import concourse.bass_interp as bi
                        out_ap = instruction.outs[0]
                        r = _orig_act(self, instruction, reg_snapshot=reg_snapshot)
                        view = self.view_ap(out_ap, bi.Direction.READ, instruction,
                                            reg_snapshot=reg_snapshot)
                        x = view.astype(np.float64)
                        view[:] = (0.5 * x * (1.0 + erf(x / np.sqrt(2.0)))).astype(view.dtype)
                        return r
                    finally:
                        instruction.func = mybir.ActivationFunctionType.Gelu
                return _orig_act(self, instruction, reg_snapshot=reg_snapshot)

            bass_interp.InstructionExecutor.visit_InstActivation = _act
            bass_interp.InstructionExecutor._gelu_patched = True
        nsim = int(os.environ.get("BASS_KERNEL_SIM_CORES", "1"))
        simtrace = bool(os.environ.get("BASS_KERNEL_SIMTRACE"))
        sim = bass_interp.MultiCoreSim(nc, nsim, trace=simtrace)
        for c in range(nsim):
            for k, v in in_maps[c].items():
                sim.cores[c].tensor(k)[:] = v
        sim.simulate()
        print(f"SIM predicted time: {sim.cores[0].time} ns")
        full = np.zeros((B, NPAD, D), np.float32)
        for c in range(nsim):
            full[:, c * NG : (c + 1) * NG, :] = (
                np.array(sim.cores[c].mem_tensor("out")).astype(np.float32)
                .reshape(B, NG, D))
        return full[:, :N, :]

    from concourse.bass_utils import run_bass_kernel_spmd
    _split_waits(nc)
    trace = bool(os.environ.get("BASS_KERNEL_TRACE"))
    res = run_bass_kernel_spmd(nc, in_maps, core_ids=list(range(NCORES)),
                               trace=trace)
    if trace and res.exec_time_ns is not None:
        print(f"HW exec time: {res.exec_time_ns} ns")
        if res.instructions_and_trace:
            print("trace:", res.instructions_and_trace[1])

    full = np.zeros((B, NPAD, D), np.float32)
    for c in range(NCORES):
        full[:, c * NG : (c + 1) * NG, :] = (
            np.asarray(res.results[c]["out"]).astype(np.float32))
    return full[:, :N, :]
